# revision 1
# baseline (speedup 1.0000x reference)
"""Builder for the 8-core DeepseekV2 decoder layer Bass kernel (v2).

Per core i (SPMD, identical program; per-core data via input shards):
  P1 seq-parallel front-end on 256 own token rows: rmsnorm -> q_a/kv_a -> latent
     norms + k_pe rope -> feature-major -> AllGather#1 [qlatT|ckvT|kpeT] (2112x256/rank).
  P2 head-parallel attention (2 heads, all 2048 tokens): q_b/kv_b feature-major,
     q_pe rope via transpose round-trip, scores^T (kt on partitions), exp without
     max-subtraction (scores O(1)), causal via skipped tiles + 4 diagonal masks,
     denominator via ones-matmul, PV, 1/den via reciprocal + broadcast-matmul.
     AllToAll: head-sharded attnT -> token-sharded full-head attnT (2048x256).
  P3 seq-parallel o_proj + residual -> r2out (ExternalOutput); ln_post ->
     AllGather#3 h2T (2048x256/rank).
  P4 TP MLP on INTER slice (1368 padded to 1408), in two T-halves for SBUF:
     gate/up -> silu*up -> down^T (hid-major) -> bf16 ReduceScatter over hid ->
     mlp_part (ExternalOutput, [256 hid, 2048 t], bf16).
Host: out = concat(r2out rows) + concat(mlp_part).T

All matmuls float32r (fp32 storage, FP22 multiply) with out free dim >= 256.
ln weights and DQK^-0.5 folded into weights host-side. Weights stream as big
strip-DMAs; per-phase scoped SBUF/PSUM pools keep the PE double-buffered.
"""
from contextlib import ExitStack

import concourse.bass as bass
import concourse.mybir as mybir
import concourse.tile as tile
from concourse import bacc
from concourse.masks import make_identity
from concourse.tile import add_dep_helper

F32 = mybir.dt.float32
F32R = mybir.dt.float32r
BF16 = mybir.dt.bfloat16
AF = mybir.ActivationFunctionType
ALU = mybir.AluOpType

T, HID, H, DN, DR, DQK, DV = 2048, 2048, 16, 128, 64, 192, 128
TH = T // 2
QLR, KVLR, INTER, EPS = 1536, 512, 10944, 1e-6
NC = 8
TPC = T // NC       # 256
HPC = H // NC       # 2
IPC = INTER // NC   # 1368
IPAD = 1408
NI = IPAD // 128    # 11
P = 128
AGW = QLR + KVLR + DR  # 2112
HK = HID // P       # 16
QK = QLR // P       # 12
KK = KVLR // P      # 4


# Enable walrus LDWEIGHTS elision (the default pipeline pins it off; verified
# numerically equivalent on this kernel).
import concourse.bass_utils as _bu
if not getattr(_bu, "_ldwopt_patched", False):
    _orig_run_command = _bu.run_command

    def _run_command_ldwopt(argv, **kw):
        argv = ["--enable-ldw-opt=true" if a == "--enable-ldw-opt=false" else a
                for a in argv]
        return _orig_run_command(argv, **kw)

    _bu.run_command = _run_command_ldwopt
    _bu._ldwopt_patched = True


def build():
    nc = bacc.Bacc("TRN2", target_bir_lowering=False, debug=False, num_devices=NC)
    rg = [list(range(NC))]

    h_rows = nc.dram_tensor("h_rows", [TPC, HID], F32, kind="ExternalInput")
    ctab_loc = nc.dram_tensor("ctab_loc", [TPC, DR // 2], F32, kind="ExternalInput")
    stab_loc = nc.dram_tensor("stab_loc", [TPC, DR // 2], F32, kind="ExternalInput")
    ctab2 = nc.dram_tensor("ctab2", [T, DR], F32, kind="ExternalInput")
    stab2 = nc.dram_tensor("stab2", [T, DR], F32, kind="ExternalInput")
    wqaT_d = nc.dram_tensor("wqaT", [HID, QLR], F32R, kind="ExternalInput")
    wkvaT_d = nc.dram_tensor("wkvaT", [HID, KVLR + DR], F32R, kind="ExternalInput")
    wqb_nope_d = nc.dram_tensor("wqb_nope", [QLR, HPC * DN], F32R, kind="ExternalInput")
    wqb_pe_d = nc.dram_tensor("wqb_pe", [QLR, HPC * DR], F32R, kind="ExternalInput")
    wkv_nope_d = nc.dram_tensor("wkv_nope", [KVLR, HPC * DN], F32R, kind="ExternalInput")
    wkv_v_d = nc.dram_tensor("wkv_v", [KVLR, HPC * DV], F32R, kind="ExternalInput")
    woT_d = nc.dram_tensor("woT", [H * DV, HID], F32R, kind="ExternalInput")
    wgT_d = nc.dram_tensor("wgT", [HID, IPAD], F32R, kind="ExternalInput")
    wuT_d = nc.dram_tensor("wuT", [HID, IPAD], F32R, kind="ExternalInput")
    wdT_d = nc.dram_tensor("wdT", [IPAD, HID], F32R, kind="ExternalInput")
    masks_d = nc.dram_tensor("masks", [4 * P, 512], F32, kind="ExternalInput")
    r2out = nc.dram_tensor("r2out", [TPC, HID], F32, kind="ExternalOutput")
    mlp_part = nc.dram_tensor("mlp_part", [HID // NC, T], BF16, kind="ExternalOutput")

    ag1q_in = nc.dram_tensor("ag1q_in", [QLR, TPC], F32R, kind="Internal")
    ag1q_out = nc.dram_tensor("ag1q_out", [NC * QLR, TPC], F32R, kind="Internal", addr_space="Shared")
    ag1k_in = nc.dram_tensor("ag1k_in", [KVLR + DR, TPC], F32R, kind="Internal")
    ag1k_out = nc.dram_tensor("ag1k_out", [NC * (KVLR + DR), TPC], F32R, kind="Internal", addr_space="Shared")
    a2a_in = nc.dram_tensor("a2a_in", [NC * HPC * DV, TPC], F32R, kind="Internal")
    a2a_out = nc.dram_tensor("a2a_out", [NC * HPC * DV, TPC], F32R, kind="Internal")
    ag3_in = nc.dram_tensor("ag3_in", [HID, TPC], F32R, kind="Internal")
    ag3_out = nc.dram_tensor("ag3_out", [NC * HID, TPC], F32R, kind="Internal", addr_space="Shared")
    rs_in_h = [nc.dram_tensor(f"rs_in{h}", [HID, T // 2], BF16, kind="Internal") for h in range(2)]
    rs_out_h = [nc.dram_tensor(f"rs_out{h}", [HID // NC, T // 2], BF16, kind="Internal") for h in range(2)]

    ctx = ExitStack()
    with tile.TileContext(nc) as tc, ctx:
        consts = ctx.enter_context(tc.tile_pool(name="consts", bufs=1))
        sb = ctx.enter_context(tc.tile_pool(name="sb", bufs=3))

        ident32 = consts.tile([P, P], F32)
        make_identity(nc, ident32[:])
        ident = consts.tile([P, P], F32R)
        nc.vector.tensor_copy(out=ident[:], in_=ident32[:])
        ones32 = consts.tile([P, 1], F32)
        nc.vector.memset(ones32[:], 1.0)
        ones_col = consts.tile([P, 1], F32R)
        nc.vector.tensor_copy(out=ones_col[:], in_=ones32[:])
        eps_col = consts.tile([P, 1], F32)
        nc.vector.memset(eps_col[:], EPS)
        ones_row = consts.tile([1, P], F32)
        nc.vector.memset(ones_row[:], 1.0)
        sqscr = consts.tile([P, HID], F32)
        mask_sb = consts.tile([P, 4, 512], F32)
        for m in range(4):
            nc.sync.dma_start(mask_sb[:, m, :], masks_d[m * P:(m + 1) * P, :])

        def rms_scale_from(ss_ap, d, tag):
            st = sb.tile([P, 1], F32, tag=f"st_{tag}")
            nc.scalar.activation(st[:], ss_ap, AF.Sqrt, bias=eps_col[:], scale=1.0 / d)
            sinv = sb.tile([P, 1], F32, tag=f"si_{tag}")
            nc.vector.reciprocal(sinv[:], st[:])
            return sinv

        def transpose_to(psT, dst_ap, src_ap, n_out, n_in=P):
            pt = psT.tile([P, P], F32R, tag="tr")
            nc.tensor.transpose(pt[:n_out, :n_in], src_ap, ident[:n_in, :n_in])
            nc.vector.tensor_copy(out=dst_ap, in_=pt[:n_out, :n_in])

        def rope_tok(x_sb_ap, c_ap, s_ap, w, tag):
            e = x_sb_ap[:, 0:w:2]
            o = x_sb_ap[:, 1:w:2]
            a1 = sb.tile([P, w // 2], F32, tag=f"ra{tag}")
            a2 = sb.tile([P, w // 2], F32, tag=f"rb{tag}")
            b1 = sb.tile([P, w // 2], F32, tag=f"rc{tag}")
            b2 = sb.tile([P, w // 2], F32, tag=f"rd{tag}")
            nc.vector.tensor_tensor(a1[:], e, c_ap, ALU.mult)
            nc.vector.tensor_tensor(a2[:], o, s_ap, ALU.mult)
            nc.vector.tensor_tensor(b1[:], o, c_ap, ALU.mult)
            nc.vector.tensor_tensor(b2[:], e, s_ap, ALU.mult)
            nc.vector.tensor_tensor(e, a1[:], a2[:], ALU.subtract)
            nc.vector.tensor_tensor(o, b1[:], b2[:], ALU.add)

        # ================= Phase 1 =================
        with tc.tile_pool(name="p1res", bufs=1) as p1res, \
             tc.tile_pool(name="p1w", bufs=2) as p1w, \
             tc.tile_pool(name="ps1", bufs=4, space="PSUM") as ps1, \
             tc.tile_pool(name="psT1", bufs=2, space="PSUM") as psT1:
            hnT = p1res.tile([P, HK, TPC], F32R)
            for tt in range(TPC // P):
                h_sb = p1res.tile([P, HID], F32, tag="h_in", bufs=2)
                nc.sync.dma_start(h_sb[:], h_rows[tt * P:(tt + 1) * P, :])
                ss = sb.tile([P, 1], F32, tag="p1ss")
                nc.scalar.activation(sqscr[:], h_sb[:], AF.Square, accum_out=ss[:])
                sinv = rms_scale_from(ss[:], HID, "p1h")
                hn = p1res.tile([P, HID], F32R, tag="hn", bufs=2)
                nc.vector.tensor_scalar_mul(hn[:], h_sb[:], sinv[:])
                for k in range(HK):
                    transpose_to(psT1, hnT[:, k, tt * P:(tt + 1) * P], hn[:, k * P:(k + 1) * P], P)

            qsb = p1res.tile([P, 2, QLR], F32R)
            ssq = p1res.tile([P, 2, 3], F32)
            for c in range(3):
                wqa_strip = p1w.tile([P, HK, 512], F32R, tag="wqa")
                nc.sync.dma_start(wqa_strip[:], wqaT_d[:, c * 512:(c + 1) * 512]
                                  .rearrange("(a p) m -> p a m", p=P))
                for tt in range(TPC // P):
                    pq = ps1.tile([P, 512], F32, tag="acc")
                    for k in range(HK):
                        nc.tensor.matmul(pq[:], hnT[:, k, tt * P:(tt + 1) * P],
                                         wqa_strip[:, k, :],
                                         start=(k == 0), stop=(k == HK - 1))
                    nc.scalar.activation(sqscr[:, :512], pq[:], AF.Square,
                                         accum_out=ssq[:, tt, c:c + 1])
                    nc.vector.tensor_copy(out=qsb[:, tt, c * 512:(c + 1) * 512], in_=pq[:])
            ksb = p1res.tile([P, 2, KVLR + DR], F32R)
            ssk = p1res.tile([P, 2, 2], F32)
            for c in range(2):
                wkva_strip = p1w.tile([P, HK, 288], F32R, tag="wkva")
                nc.sync.dma_start(wkva_strip[:], wkvaT_d[:, c * 288:(c + 1) * 288]
                                  .rearrange("(a p) m -> p a m", p=P))
                for tt in range(TPC // P):
                    pk = ps1.tile([P, 512], F32, tag="acc", name="pk")[:, :288]
                    for k in range(HK):
                        nc.tensor.matmul(pk[:], hnT[:, k, tt * P:(tt + 1) * P],
                                         wkva_strip[:, k, :],
                                         start=(k == 0), stop=(k == HK - 1))
                    w_sq = min(288, KVLR - c * 288)
                    nc.scalar.activation(sqscr[:, :w_sq], pk[:, :w_sq], AF.Square,
                                         accum_out=ssk[:, tt, c:c + 1])
                    nc.vector.tensor_copy(out=ksb[:, tt, c * 288:(c + 1) * 288], in_=pk[:])

            for tt in range(TPC // P):
                sstot = sb.tile([P, 1], F32, tag="p1qsst")
                nc.vector.reduce_sum(out=sstot[:], in_=ssq[:, tt, :], axis=mybir.AxisListType.X)
                sinv = rms_scale_from(sstot[:], QLR, "p1q")
                nc.vector.tensor_scalar_mul(qsb[:, tt, :], qsb[:, tt, :], sinv[:])
                for k in range(QK):
                    tsb = sb.tile([P, P], F32R, tag="trsb")
                    transpose_to(psT1, tsb[:], qsb[:, tt, k * P:(k + 1) * P], P)
                    nc.sync.dma_start(ag1q_in[k * P:(k + 1) * P, tt * P:(tt + 1) * P], tsb[:])
                sst = sb.tile([P, 1], F32, tag="p1ksst")
                nc.vector.reduce_sum(out=sst[:], in_=ssk[:, tt, :], axis=mybir.AxisListType.X)
                sinv = rms_scale_from(sst[:], KVLR, "p1k")
                nc.vector.tensor_scalar_mul(ksb[:, tt, :KVLR], ksb[:, tt, :KVLR], sinv[:])
                for k in range(KK):
                    tsb = sb.tile([P, P], F32R, tag="trsb")
                    transpose_to(psT1, tsb[:], ksb[:, tt, k * P:(k + 1) * P], P)
                    nc.sync.dma_start(ag1k_in[k * P:(k + 1) * P,
                                              tt * P:(tt + 1) * P], tsb[:])
                c_sb = sb.tile([P, DR // 2], F32, tag="p1c")
                s_sb = sb.tile([P, DR // 2], F32, tag="p1s")
                nc.sync.dma_start(c_sb[:], ctab_loc[tt * P:(tt + 1) * P, :])
                nc.sync.dma_start(s_sb[:], stab_loc[tt * P:(tt + 1) * P, :])
                rope_tok(ksb[:, tt, KVLR:KVLR + DR], c_sb[:], s_sb[:], DR, "p1")
                tkp = sb.tile([DR, P], F32R, tag="tkp")
                transpose_to(psT1, tkp[:], ksb[:, tt, KVLR:KVLR + DR], DR)
                nc.sync.dma_start(ag1k_in[KVLR:KVLR + DR, tt * P:(tt + 1) * P], tkp[:])

        cc1k = nc.gpsimd.collective_compute(
            "AllGather", ALU.bypass, replica_groups=rg,
            ins=[ag1k_in[:].opt()], outs=[ag1k_out[:].opt()])
        cc1 = nc.gpsimd.collective_compute(
            "AllGather", ALU.bypass, replica_groups=rg,
            ins=[ag1q_in[:].opt()], outs=[ag1q_out[:].opt()])
        add_dep_helper(cc1.ins, cc1k.ins, sync=True, reason="serialize collectives")

        # ================= Phase 2 =================
        with tc.tile_pool(name="attn", bufs=1) as at, \
             tc.tile_pool(name="ps2", bufs=4, space="PSUM") as ps2, \
             tc.tile_pool(name="psD", bufs=1, space="PSUM") as psD, \
             tc.tile_pool(name="psB", bufs=1, space="PSUM") as psB, \
             tc.tile_pool(name="psT2", bufs=2, space="PSUM") as psT2:
            qnT = at.tile([P, HPC, T], F32R)
            knT = at.tile([P, HPC, T], F32R)
            qpT = at.tile([DR, HPC, T], F32R)
            vtok = at.tile([P, T // P, HPC, DV], F32R)
            kpeT = at.tile([DR, T], F32R)
            qpT_flat = at.tile([P, T], F32R)
            wqbn_sb = at.tile([P, QK, HPC * DN], F32R)
            wqbp_sb = at.tile([P, QK, HPC * DR], F32R)
            wkvn_sb = at.tile([P, KK, HPC * DN], F32R)
            wkvv_sb = at.tile([P, KK, HPC * DV], F32R)
            nc.sync.dma_start(wqbn_sb[:], wqb_nope_d[:].rearrange("(a p) m -> p a m", p=P))
            nc.sync.dma_start(wqbp_sb[:], wqb_pe_d[:].rearrange("(a p) m -> p a m", p=P))
            nc.sync.dma_start(wkvn_sb[:], wkv_nope_d[:].rearrange("(a p) m -> p a m", p=P))
            nc.sync.dma_start(wkvv_sb[:], wkv_v_d[:].rearrange("(a p) m -> p a m", p=P))
            KW = KVLR + DR
            for b in range(NC):
                nc.sync.dma_start(kpeT[:, b * TPC:(b + 1) * TPC],
                                  ag1k_out[b * KW + KVLR:b * KW + KW, :])

            for bp in range(NC // 2):
                cs = slice(bp * 2 * TPC, (bp + 1) * 2 * TPC)
                qlat_sb = at.tile([P, QK, 2 * TPC], F32R, tag="qlat_b", bufs=1)
                ckv_sb = at.tile([P, KK, 2 * TPC], F32R, tag="ckv_b", bufs=1)
                for w in range(2):
                    b = 2 * bp + w
                    nc.sync.dma_start(qlat_sb[:, :, w * TPC:(w + 1) * TPC],
                                      ag1q_out[b * QLR:(b + 1) * QLR, :]
                                      .rearrange("(a p) m -> p a m", p=P))
                    nc.sync.dma_start(ckv_sb[:, :, w * TPC:(w + 1) * TPC],
                                      ag1k_out[b * KW:b * KW + KVLR, :]
                                      .rearrange("(a p) m -> p a m", p=P))
                for hh in range(HPC):
                    pn = ps2.tile([P, 2 * TPC], F32, tag="acc")
                    for k in range(QK):
                        nc.tensor.matmul(pn[:], wqbn_sb[:, k, hh * DN:(hh + 1) * DN],
                                         qlat_sb[:, k, :], start=(k == 0), stop=(k == QK - 1))
                    nc.vector.tensor_copy(out=qnT[:, hh, cs], in_=pn[:])
                    pkn = ps2.tile([P, 2 * TPC], F32, tag="acc")
                    for k in range(KK):
                        nc.tensor.matmul(pkn[:], wkvn_sb[:, k, hh * DN:(hh + 1) * DN],
                                         ckv_sb[:, k, :], start=(k == 0), stop=(k == KK - 1))
                    nc.vector.tensor_copy(out=knT[:, hh, cs], in_=pkn[:])
                    pv = ps2.tile([P, 2 * TPC], F32, tag="acc")
                    for k in range(KK):
                        nc.tensor.matmul(pv[:], wkvv_sb[:, k, hh * DV:(hh + 1) * DV],
                                         ckv_sb[:, k, :], start=(k == 0), stop=(k == KK - 1))
                    vT_sb = at.tile([P, 2 * TPC], F32R, tag="vT_sb", bufs=2)
                    nc.vector.tensor_copy(out=vT_sb[:], in_=pv[:])
                    for tq in range(2 * TPC // P):
                        transpose_to(psT2, vtok[:, bp * 4 + tq, hh, :],
                                     vT_sb[:, tq * P:(tq + 1) * P], P)
                pqp = ps2.tile([P, 2 * TPC], F32, tag="acc")
                for k in range(QK):
                    nc.tensor.matmul(pqp[:], wqbp_sb[:, k, :], qlat_sb[:, k, :],
                                     start=(k == 0), stop=(k == QK - 1))
                nc.vector.tensor_copy(out=qpT_flat[:, cs], in_=pqp[:])

            for tt in range(T // P):
                qp_tok = sb.tile([P, P], F32R, tag="qp_tok")
                transpose_to(psT2, qp_tok[:], qpT_flat[:, tt * P:(tt + 1) * P], P)
                c_sb = sb.tile([P, DR], F32, tag="p2c")
                s_sb = sb.tile([P, DR], F32, tag="p2s")
                nc.sync.dma_start(c_sb[:], ctab2[tt * P:(tt + 1) * P, :])
                nc.sync.dma_start(s_sb[:], stab2[tt * P:(tt + 1) * P, :])
                rope_tok(qp_tok[:], c_sb[:], s_sb[:], P, "p2")
                for hh in range(HPC):
                    transpose_to(psT2, qpT[:, hh, tt * P:(tt + 1) * P],
                                 qp_tok[:, hh * DR:(hh + 1) * DR], DR)

            NQ = 512
            for hh in range(HPC):
                for j in range(T // NQ):
                    qs = slice(j * NQ, (j + 1) * NQ)
                    u_ps = ps2.tile([P, NQ], F32, tag="acc")
                    den_ps = psD.tile([1, NQ], F32, tag="den")
                    n_kt = 4 * (j + 1)
                    for i in range(n_kt):
                        ks = slice(i * P, (i + 1) * P)
                        s_ps = ps2.tile([P, NQ], F32, tag="acc")
                        nc.tensor.matmul(s_ps[:], knT[:, hh, ks], qnT[:, hh, qs],
                                         start=True, stop=False)
                        nc.tensor.matmul(s_ps[:], kpeT[:, ks], qpT[:, hh, qs],
                                         start=False, stop=True)
                        e_sb = at.tile([P, NQ], F32R, tag="e_sb", bufs=3)
                        nc.scalar.activation(e_sb[:], s_ps[:], AF.Exp)
                        if i >= 4 * j:
                            nc.vector.tensor_tensor(e_sb[:], e_sb[:],
                                                    mask_sb[:, i - 4 * j, :], ALU.mult)
                        nc.tensor.matmul(den_ps[:], ones_col[:], e_sb[:],
                                         start=(i == 0), stop=(i == n_kt - 1),
                                         skip_group_check=True)
                        nc.tensor.matmul(u_ps[:], vtok[:, i, hh, :], e_sb[:],
                                         start=(i == 0), stop=(i == n_kt - 1),
                                         skip_group_check=True)
                    rden = at.tile([1, NQ], F32, tag="rden", bufs=2)
                    nc.vector.reciprocal(rden[:], den_ps[:])
                    bc_ps = psB.tile([P, NQ], F32, tag="bc")
                    nc.tensor.matmul(bc_ps[:], ones_row[:], rden[:], start=True, stop=True)
                    bc_sb = at.tile([P, NQ], F32, tag="bc_sb", bufs=2)
                    nc.vector.tensor_copy(out=bc_sb[:], in_=bc_ps[:])
                    a_sb = at.tile([P, NQ], F32R, tag="a_sb", bufs=2)
                    nc.vector.tensor_tensor(a_sb[:], u_ps[:], bc_sb[:], ALU.mult)
                    for w in range(2):
                        tw = 2 * j + w
                        nc.sync.dma_start(
                            a2a_in[tw * HPC * DV + hh * DV: tw * HPC * DV + (hh + 1) * DV, :],
                            a_sb[:, w * TPC:(w + 1) * TPC])

        cc2 = nc.gpsimd.collective_compute(
            "AllToAll", ALU.bypass, replica_groups=rg,
            ins=[a2a_in[:].opt()], outs=[a2a_out[:].opt()])
        add_dep_helper(cc2.ins, cc1.ins, sync=True, reason="serialize collectives")

        # ================= Phase 3 =================
        with tc.tile_pool(name="p3", bufs=1) as p3, \
             tc.tile_pool(name="p3w", bufs=2) as p3w, \
             tc.tile_pool(name="ps3", bufs=4, space="PSUM") as ps3, \
             tc.tile_pool(name="psT3", bufs=2, space="PSUM") as psT3:
            a_slabs, h_sbs, r2sbs = [], [], []
            for tt in range(TPC // P):
                a_slab = p3.tile([P, HK, P], F32R, tag=f"a_slab{tt}")
                nc.sync.dma_start(a_slab[:],
                                  a2a_out[:, tt * P:(tt + 1) * P]
                                  .rearrange("(a p) m -> p a m", p=P))
                a_slabs.append(a_slab)
                h_sb = p3.tile([P, HID], F32, tag=f"h_in3{tt}")
                nc.sync.dma_start(h_sb[:], h_rows[tt * P:(tt + 1) * P, :])
                h_sbs.append(h_sb)
                r2sbs.append(p3.tile([P, HID], F32, tag=f"r2sb{tt}", name="r2sb"))
            for hc in range(4):
                wo_strip = p3w.tile([P, HK, 512], F32R, tag="wo")
                nc.sync.dma_start(wo_strip[:], woT_d[:, hc * 512:(hc + 1) * 512]
                                  .rearrange("(a p) m -> p a m", p=P))
                for tt in range(TPC // P):
                    po = ps3.tile([P, 512], F32, tag="acc")
                    for k in range(HK):
                        nc.tensor.matmul(po[:], a_slabs[tt][:, k, :], wo_strip[:, k, :],
                                         start=(k == 0), stop=(k == HK - 1))
                    nc.vector.tensor_tensor(r2sbs[tt][:, hc * 512:(hc + 1) * 512], po[:],
                                            h_sbs[tt][:, hc * 512:(hc + 1) * 512], ALU.add)
            for tt in range(TPC // P):
                r2sb = r2sbs[tt]
                nc.sync.dma_start(r2out[tt * P:(tt + 1) * P, :], r2sb[:])
                ss = sb.tile([P, 1], F32, tag="p3ss")
                nc.scalar.activation(sqscr[:], r2sb[:], AF.Square, accum_out=ss[:])
                sinv = rms_scale_from(ss[:], HID, "p3")
                h2n = p3.tile([P, HID], F32R, tag="h2n", bufs=2)
                nc.vector.tensor_scalar_mul(h2n[:], r2sb[:], sinv[:])
                for k in range(HK):
                    tsb = sb.tile([P, P], F32R, tag="trsb")
                    transpose_to(psT3, tsb[:], h2n[:, k * P:(k + 1) * P], P)
                    nc.sync.dma_start(ag3_in[k * P:(k + 1) * P, tt * P:(tt + 1) * P], tsb[:])

        cc3 = nc.gpsimd.collective_compute(
            "AllGather", ALU.bypass, replica_groups=rg,
            ins=[ag3_in[:].opt()], outs=[ag3_out[:].opt()])
        add_dep_helper(cc3.ins, cc2.ins, sync=True, reason="serialize collectives")

        # ================= Phase 4: MLP in two T-halves =================
        with tc.tile_pool(name="p4", bufs=1) as p4, \
             tc.tile_pool(name="p4w", bufs=2) as p4w, \
             tc.tile_pool(name="ps4", bufs=8, space="PSUM") as ps4:
            for half in range(2):
                h2T = p4.tile([P, HK, TH], F32R, tag="h2T")
                for k in range(HK):
                    for bb in range(4):
                        b = half * 4 + bb
                        nc.sync.dma_start(h2T[:, k, bb * TPC:(bb + 1) * TPC],
                                          ag3_out[b * HID + k * P: b * HID + (k + 1) * P, :])
                mT = p4.tile([P, NI, TH], F32R, tag="mT")
                for m in range(NI):
                    wg_strip = p4w.tile([P, HK, P], F32R, tag="wg")
                    nc.sync.dma_start(wg_strip[:], wgT_d[:, m * P:(m + 1) * P]
                                      .rearrange("(a p) m -> p a m", p=P))
                    wu_strip = p4w.tile([P, HK, P], F32R, tag="wu")
                    nc.sync.dma_start(wu_strip[:], wuT_d[:, m * P:(m + 1) * P]
                                      .rearrange("(a p) m -> p a m", p=P))
                    pg = [ps4.tile([P, 512], F32, tag="acc", name=f"pg{_}") for _ in range(2)]
                    pu = [ps4.tile([P, 512], F32, tag="acc", name=f"pu{_}") for _ in range(2)]
                    for k in range(HK):
                        for ts in range(2):
                            nc.tensor.matmul(pg[ts][:], wg_strip[:, k, :],
                                             h2T[:, k, ts * 512:(ts + 1) * 512],
                                             start=(k == 0), stop=(k == HK - 1),
                                             skip_group_check=True)
                        for ts in range(2):
                            nc.tensor.matmul(pu[ts][:], wu_strip[:, k, :],
                                             h2T[:, k, ts * 512:(ts + 1) * 512],
                                             start=(k == 0), stop=(k == HK - 1),
                                             skip_group_check=True)
                    for ts in range(2):
                        sg = p4.tile([P, 512], F32, tag="sg", bufs=2)
                        nc.scalar.activation(sg[:], pg[ts][:], AF.Silu)
                        nc.vector.tensor_tensor(mT[:, m, ts * 512:(ts + 1) * 512],
                                                sg[:], pu[ts][:], ALU.mult)
                for hm in range(HK):
                    wd_strip = p4w.tile([P, NI, P], F32R, tag="wd")
                    nc.sync.dma_start(wd_strip[:], wdT_d[:, hm * P:(hm + 1) * P]
                                      .rearrange("(a p) m -> p a m", p=P))
                    pd = [ps4.tile([P, 512], F32, tag="acc", name=f"pd{_}") for _ in range(2)]
                    for m in range(NI):
                        for ts in range(2):
                            nc.tensor.matmul(pd[ts][:], wd_strip[:, m, :],
                                             mT[:, m, ts * 512:(ts + 1) * 512],
                                             start=(m == 0), stop=(m == NI - 1),
                                             skip_group_check=True)
                    dsb = p4.tile([P, TH], BF16, tag="dsb", bufs=2)
                    for ts in range(2):
                        nc.vector.tensor_copy(out=dsb[:, ts * 512:(ts + 1) * 512], in_=pd[ts][:])
                    nc.sync.dma_start(rs_in_h[half][hm * P:(hm + 1) * P, :], dsb[:])

        cc_prev = cc3
        for h in range(2):
            cc4 = nc.gpsimd.collective_compute(
                "ReduceScatter", ALU.add, replica_groups=rg,
                ins=[rs_in_h[h][:].opt()], outs=[rs_out_h[h][:].opt()])
            add_dep_helper(cc4.ins, cc_prev.ins, sync=True, reason="serialize collectives")
            cc_prev = cc4
            nc.sync.dma_start(mlp_part[:, h * TH:(h + 1) * TH], rs_out_h[h][:])

    nc.finalize()
    return nc


# ===================== host side =====================
import numpy as np

_nc_cache = None


def _get_nc():
    global _nc_cache
    if _nc_cache is None:
        _nc_cache = build()
    return _nc_cache


def prep_shards(inputs):
    f = lambda x: np.ascontiguousarray(np.asarray(x, np.float32))
    hs = f(inputs["hidden_states"])
    cos = f(inputs["cos"])[:, 0, :]
    sin = f(inputs["sin"])[:, 0, :]
    ctab = cos[:, :DR // 2]                     # (T, 32)
    stab = sin[:, :DR // 2]
    ctab2 = np.concatenate([ctab, ctab], 1)     # (T, 64) for 2-head q rope
    stab2 = np.concatenate([stab, stab], 1)
    ln_in = f(inputs["ln_input"])
    wqaT = f((inputs["w_q_a"] * ln_in[None, :]).T)
    wkvaT = f((inputs["w_kv_a"] * ln_in[None, :]).T)
    wqbT = f((inputs["w_q_b"] * f(inputs["ln_q_a"])[None, :]).T) * np.float32(DQK ** -0.5)
    wkvbT = f((inputs["w_kv_b"] * f(inputs["ln_kv_a"])[None, :]).T)
    woT = f(inputs["w_o"].T)
    ln_post = f(inputs["ln_post"])
    wgT = f((inputs["w_gate"] * ln_post[None, :]).T)
    wuT = f((inputs["w_up"] * ln_post[None, :]).T)
    wdT = f(inputs["w_down"].T)

    # causal diagonal masks: variant m valid iff 128*m + kt_local <= qt_local
    masks = np.zeros((4, P, 512), np.float32)
    kt = np.arange(P)[:, None]
    qt = np.arange(512)[None, :]
    for m in range(4):
        masks[m] = (P * m + kt <= qt).astype(np.float32)
    masks = masks.reshape(4 * P, 512)

    # per-head column split of wqbT (QLR, H*DQK): head h cols [h*DQK, (h+1)*DQK)
    wqb3 = wqbT.reshape(QLR, H, DQK)
    wkv3 = wkvbT.reshape(KVLR, H, DN + DV)
    shards = []
    for i in range(NC):
        hsl = slice(i * HPC, (i + 1) * HPC)
        wqb_nope = np.ascontiguousarray(wqb3[:, hsl, :DN].reshape(QLR, HPC * DN))
        wqb_pe = np.ascontiguousarray(wqb3[:, hsl, DN:].reshape(QLR, HPC * DR))
        wkv_nope = np.ascontiguousarray(wkv3[:, hsl, :DN].reshape(KVLR, HPC * DN))
        wkv_v = np.ascontiguousarray(wkv3[:, hsl, DN:].reshape(KVLR, HPC * DV))
        wg_s = np.zeros((HID, IPAD), np.float32)
        wg_s[:, :IPC] = wgT[:, i * IPC:(i + 1) * IPC]
        wu_s = np.zeros((HID, IPAD), np.float32)
        wu_s[:, :IPC] = wuT[:, i * IPC:(i + 1) * IPC]
        wd_s = np.zeros((IPAD, HID), np.float32)
        wd_s[:IPC, :] = wdT[i * IPC:(i + 1) * IPC, :]
        shards.append({
            "h_rows": np.ascontiguousarray(hs[i * TPC:(i + 1) * TPC]),
            "ctab_loc": np.ascontiguousarray(ctab[i * TPC:(i + 1) * TPC]),
            "stab_loc": np.ascontiguousarray(stab[i * TPC:(i + 1) * TPC]),
            "ctab2": ctab2, "stab2": stab2,
            "wqaT": wqaT, "wkvaT": wkvaT,
            "wqb_nope": wqb_nope, "wqb_pe": wqb_pe,
            "wkv_nope": wkv_nope, "wkv_v": wkv_v,
            "woT": woT, "wgT": wg_s, "wuT": wu_s, "wdT": wd_s,
            "masks": masks,
        })
    return shards


def kernel(**inputs):
    from concourse.bass_utils import run_bass_kernel_spmd
    nc = _get_nc()
    shards = prep_shards(inputs)
    res = run_bass_kernel_spmd(nc, shards, core_ids=list(range(NC)))
    return assemble(res.results)


def assemble(results):
    r2 = np.concatenate([results[i]["r2out"] for i in range(NC)], axis=0)      # (T, HID)
    mlpT = np.concatenate([results[i]["mlp_part"] for i in range(NC)], axis=0).astype(np.float32)  # (HID, T)
    return r2 + mlpT.T



# revision 6
# speedup vs baseline: 1.2375x; 1.2375x over previous
"""Builder for the 8-core DeepseekV2 decoder layer Bass kernel (v3).

Restructured from v2 for collective/compute overlap and PE p-state:
  P1 seq-parallel front-end on 256 own rows: rmsnorm -> kv_a -> latent norm +
     k_pe rope -> AllGather#1k [ckvT|kpeT] (576x256/rank) launched EARLY;
     then q_a -> qnorm -> q_b (all 16 heads, token-local) -> q rope ->
     AllToAll-q (head-sharded qT, bf16, 4x fewer bytes than v2's AllGather-q).
  P2 head-parallel attention (2 heads, all 2048 tokens): kv_b feature-major,
     scores^T with bf16 q moving, exp without max-subtraction, causal via
     skipped tiles + 4 diagonal masks, den via ones-matmul, PV, reciprocal +
     broadcast-matmul. Attn-out AllToAll SPLIT BY HEAD: head 0's A2A overlaps
     head 1's compute. o_proj weights (bf16) preloaded to SBUF during P2.
  P3 seq-parallel o_proj + residual -> r2out; ln_post -> h2T bf16, AllGather#3
     chunked per 128-token block (bf16, 4x fewer bytes than v2).
  P4 TP MLP on INTER slice, single pass over all T in bf16 (weights streamed
     once); down^T per 512-token chunk -> chunked bf16 ReduceScatter overlapped
     with the next chunk's compute; only the last RS chunk is exposed.
Host: out = concat(r2out rows) + concat(mlp_part).T

All matmuls fp32r or bf16 (1 cycle/row at free>=256), f32 PSUM. ln weights and
DQK^-0.5 folded into weights host-side (bf16).
"""
from contextlib import ExitStack

import concourse.bass as bass
import concourse.mybir as mybir
import concourse.tile as tile
from concourse import bacc
from concourse.masks import make_identity
from concourse.tile import add_dep_helper

F32 = mybir.dt.float32
F32R = mybir.dt.float32r
BF16 = mybir.dt.bfloat16
AF = mybir.ActivationFunctionType
ALU = mybir.AluOpType

T, HID, H, DN, DR, DQK, DV = 2048, 2048, 16, 128, 64, 192, 128
QLR, KVLR, INTER, EPS = 1536, 512, 10944, 1e-6
NC = 8
TPC = T // NC       # 256
HPC = H // NC       # 2
IPC = INTER // NC   # 1368
IPAD = 1408
NI = IPAD // 128    # 11
P = 128
AGK = KVLR + DR     # 576
HK = HID // P       # 16
QK = QLR // P       # 12
KK = KVLR // P      # 4
NFC = H * DN // P   # 16 nope feature chunks of q
PFC = H * DR // P   # 8 pe feature chunks of q
DST = HPC * DQK     # 384 rows per dest in a2a-q


# NOTE: v2 enabled walrus LDWEIGHTS elision (--enable-ldw-opt=true), but that
# optimization rejects some bf16 LDWEIGHTS patterns in this kernel
# ("InstLdweights is not compatible with LDW optimization"), so v3 runs with
# the default. LDWEIGHTS issues on its own queue and overlaps MATMUL.


def build():
    nc = bacc.Bacc("TRN2", target_bir_lowering=False, debug=False, num_devices=NC)
    rg = [list(range(NC))]

    h_rows = nc.dram_tensor("h_rows", [TPC, HID], F32, kind="ExternalInput")
    ctab_loc = nc.dram_tensor("ctab_loc", [TPC, DR // 2], F32, kind="ExternalInput")
    stab_loc = nc.dram_tensor("stab_loc", [TPC, DR // 2], F32, kind="ExternalInput")
    ctab2_loc = nc.dram_tensor("ctab2_loc", [TPC, DR], F32, kind="ExternalInput")
    stab2_loc = nc.dram_tensor("stab2_loc", [TPC, DR], F32, kind="ExternalInput")
    wqaT_d = nc.dram_tensor("wqaT", [HID, QLR], BF16, kind="ExternalInput")
    wkvaT_d = nc.dram_tensor("wkvaT", [HID, AGK], BF16, kind="ExternalInput")
    wqbnT_d = nc.dram_tensor("wqbnT", [QLR, H * DN], BF16, kind="ExternalInput")
    wqbpT_d = nc.dram_tensor("wqbpT", [QLR, H * DR], BF16, kind="ExternalInput")
    wkv_nope_d = nc.dram_tensor("wkv_nope", [KVLR, HPC * DN], BF16, kind="ExternalInput")
    wkv_v_d = nc.dram_tensor("wkv_v", [KVLR, HPC * DV], BF16, kind="ExternalInput")
    woT_d = nc.dram_tensor("woT", [H * DV, HID], BF16, kind="ExternalInput")
    wgT_d = nc.dram_tensor("wgT", [HID, IPAD], BF16, kind="ExternalInput")
    wuT_d = nc.dram_tensor("wuT", [HID, IPAD], BF16, kind="ExternalInput")
    wdT_d = nc.dram_tensor("wdT", [IPAD, HID], BF16, kind="ExternalInput")
    masks_d = nc.dram_tensor("masks", [4 * P, 512], F32, kind="ExternalInput")
    r2out = nc.dram_tensor("r2out", [TPC, HID], F32, kind="ExternalOutput")
    mlp_part = nc.dram_tensor("mlp_part", [HID // NC, T], BF16, kind="ExternalOutput")

    ag1k_in = nc.dram_tensor("ag1k_in", [AGK, TPC], BF16, kind="Internal")
    ag1k_out = nc.dram_tensor("ag1k_out", [NC * AGK, TPC], BF16, kind="Internal", addr_space="Shared")
    a2aq_in = nc.dram_tensor("a2aq_in", [NC * DST, TPC], BF16, kind="Internal")
    a2aq_out = nc.dram_tensor("a2aq_out", [NC * DST, TPC], BF16, kind="Internal")
    a2aA_in = nc.dram_tensor("a2aA_in", [NC * DV, TPC], BF16, kind="Internal")
    a2aA_out = nc.dram_tensor("a2aA_out", [NC * DV, TPC], BF16, kind="Internal")
    a2aB_in = nc.dram_tensor("a2aB_in", [NC * DV, TPC], BF16, kind="Internal")
    a2aB_out = nc.dram_tensor("a2aB_out", [NC * DV, TPC], BF16, kind="Internal")
    ag3_in_t = [nc.dram_tensor(f"ag3_in{t}", [HID, P], BF16, kind="Internal") for t in range(2)]
    ag3_out_t = [nc.dram_tensor(f"ag3_out{t}", [NC * HID, P], BF16, kind="Internal", addr_space="Shared") for t in range(2)]
    rs_in_c = [nc.dram_tensor(f"rs_in{c}", [HID, 512], BF16, kind="Internal") for c in range(4)]
    rs_out_c = [nc.dram_tensor(f"rs_out{c}", [HID // NC, 512], BF16, kind="Internal") for c in range(4)]

    ctx = ExitStack()
    with tile.TileContext(nc) as tc, ctx:
        consts = ctx.enter_context(tc.tile_pool(name="consts", bufs=1))
        sb = ctx.enter_context(tc.tile_pool(name="sb", bufs=3))

        ident32 = consts.tile([P, P], F32)
        make_identity(nc, ident32[:])
        ident = consts.tile([P, P], F32R)
        nc.vector.tensor_copy(out=ident[:], in_=ident32[:])
        ones32 = consts.tile([P, 1], F32)
        nc.vector.memset(ones32[:], 1.0)
        ones_col = consts.tile([P, 1], F32R)
        nc.vector.tensor_copy(out=ones_col[:], in_=ones32[:])
        eps_col = consts.tile([P, 1], F32)
        nc.vector.memset(eps_col[:], EPS)
        ones_row = consts.tile([1, P], F32)
        nc.vector.memset(ones_row[:], 1.0)
        sqscr = consts.tile([P, HID], F32)
        mask_sb = consts.tile([P, 4, 512], F32)
        for m in range(4):
            nc.sync.dma_start(mask_sb[:, m, :], masks_d[m * P:(m + 1) * P, :])

        def rms_scale_from(ss_ap, d, tag):
            st = sb.tile([P, 1], F32, tag=f"st_{tag}")
            nc.scalar.activation(st[:], ss_ap, AF.Sqrt, bias=eps_col[:], scale=1.0 / d)
            sinv = sb.tile([P, 1], F32, tag=f"si_{tag}")
            nc.vector.reciprocal(sinv[:], st[:])
            return sinv

        def transpose_to(psT, dst_ap, src_ap, n_out, n_in=P):
            pt = psT.tile([P, P], F32R, tag="tr")
            nc.tensor.transpose(pt[:n_out, :n_in], src_ap, ident[:n_in, :n_in])
            nc.vector.tensor_copy(out=dst_ap, in_=pt[:n_out, :n_in])

        def rope_tok(x_sb_ap, c_ap, s_ap, w, tag):
            e = x_sb_ap[:, 0:w:2]
            o = x_sb_ap[:, 1:w:2]
            a1 = sb.tile([P, w // 2], F32, tag=f"ra{tag}")
            a2 = sb.tile([P, w // 2], F32, tag=f"rb{tag}")
            b1 = sb.tile([P, w // 2], F32, tag=f"rc{tag}")
            b2 = sb.tile([P, w // 2], F32, tag=f"rd{tag}")
            nc.vector.tensor_tensor(a1[:], e, c_ap, ALU.mult)
            nc.vector.tensor_tensor(a2[:], o, s_ap, ALU.mult)
            nc.vector.tensor_tensor(b1[:], o, c_ap, ALU.mult)
            nc.vector.tensor_tensor(b2[:], e, s_ap, ALU.mult)
            nc.vector.tensor_tensor(e, a1[:], a2[:], ALU.subtract)
            nc.vector.tensor_tensor(o, b1[:], b2[:], ALU.add)

        # ================= Phase 1 =================
        with tc.tile_pool(name="p1res", bufs=1) as p1res, \
             tc.tile_pool(name="p1w", bufs=2) as p1w, \
             tc.tile_pool(name="ps1", bufs=4, space="PSUM") as ps1, \
             tc.tile_pool(name="psT1", bufs=2, space="PSUM") as psT1:
            hnT = p1res.tile([P, HK, TPC], BF16)
            for tt in range(TPC // P):
                h_sb = p1res.tile([P, HID], F32, tag="h_in", bufs=2)
                nc.sync.dma_start(h_sb[:], h_rows[tt * P:(tt + 1) * P, :])
                ss = sb.tile([P, 1], F32, tag="p1ss")
                nc.scalar.activation(sqscr[:], h_sb[:], AF.Square, accum_out=ss[:])
                sinv = rms_scale_from(ss[:], HID, "p1h")
                hn = p1res.tile([P, HID], F32R, tag="hn", bufs=2)
                nc.vector.tensor_scalar_mul(hn[:], h_sb[:], sinv[:])
                for k in range(HK):
                    transpose_to(psT1, hnT[:, k, tt * P:(tt + 1) * P], hn[:, k * P:(k + 1) * P], P)

            # --- kv_a first so AllGather#1k launches early ---
            ksb = p1res.tile([P, 2, AGK], F32R)
            ssk = p1res.tile([P, 2, 2], F32)
            for c in range(2):
                wkva_strip = p1w.tile([P, HK, 288], BF16, tag="wkva")
                nc.sync.dma_start(wkva_strip[:], wkvaT_d[:, c * 288:(c + 1) * 288]
                                  .rearrange("(a p) m -> p a m", p=P))
                for tt in range(TPC // P):
                    pk = ps1.tile([P, 512], F32, tag="acc", name="pk")[:, :288]
                    for k in range(HK):
                        nc.tensor.matmul(pk[:], hnT[:, k, tt * P:(tt + 1) * P],
                                         wkva_strip[:, k, :],
                                         start=(k == 0), stop=(k == HK - 1))
                    w_sq = min(288, KVLR - c * 288)
                    nc.scalar.activation(sqscr[:, :w_sq], pk[:, :w_sq], AF.Square,
                                         accum_out=ssk[:, tt, c:c + 1])
                    nc.vector.tensor_copy(out=ksb[:, tt, c * 288:(c + 1) * 288], in_=pk[:])
            for tt in range(TPC // P):
                sst = sb.tile([P, 1], F32, tag="p1ksst")
                nc.vector.reduce_sum(out=sst[:], in_=ssk[:, tt, :], axis=mybir.AxisListType.X)
                sinv = rms_scale_from(sst[:], KVLR, "p1k")
                nc.vector.tensor_scalar_mul(ksb[:, tt, :KVLR], ksb[:, tt, :KVLR], sinv[:])
                for k in range(KK):
                    tsb = sb.tile([P, P], BF16, tag="trsb")
                    transpose_to(psT1, tsb[:], ksb[:, tt, k * P:(k + 1) * P], P)
                    nc.sync.dma_start(ag1k_in[k * P:(k + 1) * P,
                                              tt * P:(tt + 1) * P], tsb[:])
                c_sb = sb.tile([P, DR // 2], F32, tag="p1c")
                s_sb = sb.tile([P, DR // 2], F32, tag="p1s")
                nc.sync.dma_start(c_sb[:], ctab_loc[tt * P:(tt + 1) * P, :])
                nc.sync.dma_start(s_sb[:], stab_loc[tt * P:(tt + 1) * P, :])
                rope_tok(ksb[:, tt, KVLR:KVLR + DR], c_sb[:], s_sb[:], DR, "p1")
                tkp = sb.tile([DR, P], BF16, tag="tkp")
                transpose_to(psT1, tkp[:], ksb[:, tt, KVLR:KVLR + DR], DR)
                nc.sync.dma_start(ag1k_in[KVLR:KVLR + DR, tt * P:(tt + 1) * P], tkp[:])

            cc1k = nc.gpsimd.collective_compute(
                "AllGather", ALU.bypass, replica_groups=rg,
                ins=[ag1k_in[:].opt()], outs=[ag1k_out[:].opt()])

            # --- q path: q_a -> qnorm -> qlatT (SBUF) ---
            qsb = p1res.tile([P, 2, QLR], F32R)
            ssq = p1res.tile([P, 2, 3], F32)
            for c in range(3):
                wqa_strip = p1w.tile([P, HK, 512], BF16, tag="wqa")
                nc.sync.dma_start(wqa_strip[:], wqaT_d[:, c * 512:(c + 1) * 512]
                                  .rearrange("(a p) m -> p a m", p=P))
                for tt in range(TPC // P):
                    pq = ps1.tile([P, 512], F32, tag="acc")
                    for k in range(HK):
                        nc.tensor.matmul(pq[:], hnT[:, k, tt * P:(tt + 1) * P],
                                         wqa_strip[:, k, :],
                                         start=(k == 0), stop=(k == HK - 1))
                    nc.scalar.activation(sqscr[:, :512], pq[:], AF.Square,
                                         accum_out=ssq[:, tt, c:c + 1])
                    nc.vector.tensor_copy(out=qsb[:, tt, c * 512:(c + 1) * 512], in_=pq[:])
            qlatT = p1res.tile([P, QK, TPC], BF16)
            for tt in range(TPC // P):
                sstot = sb.tile([P, 1], F32, tag="p1qsst")
                nc.vector.reduce_sum(out=sstot[:], in_=ssq[:, tt, :], axis=mybir.AxisListType.X)
                sinv = rms_scale_from(sstot[:], QLR, "p1q")
                nc.vector.tensor_scalar_mul(qsb[:, tt, :], qsb[:, tt, :], sinv[:])
                for k in range(QK):
                    transpose_to(psT1, qlatT[:, k, tt * P:(tt + 1) * P],
                                 qsb[:, tt, k * P:(k + 1) * P], P)

            # --- q_b nope: feature-major per head, straight to a2aq_in ---
            for fc in range(NFC):
                wstrip = p1w.tile([P, QK, P], BF16, tag="wqbn")
                nc.sync.dma_start(wstrip[:], wqbnT_d[:, fc * P:(fc + 1) * P]
                                  .rearrange("(a p) m -> p a m", p=P))
                pn = ps1.tile([P, 512], F32, tag="acc", name="pn")[:, :TPC]
                for k in range(QK):
                    nc.tensor.matmul(pn[:], wstrip[:, k, :], qlatT[:, k, :],
                                     start=(k == 0), stop=(k == QK - 1))
                qn_bf = sb.tile([P, TPC], BF16, tag="qnbf")
                nc.vector.tensor_copy(out=qn_bf[:], in_=pn[:])
                base = (fc // 2) * DST + (fc % 2) * DQK
                nc.sync.dma_start(a2aq_in[base:base + P, :], qn_bf[:])
            # --- q_b pe: feature-major, rope via transpose round-trip ---
            qpeT = p1res.tile([P, PFC, TPC], F32R)
            for fp in range(PFC):
                wstrip = p1w.tile([P, QK, P], BF16, tag="wqbp")
                nc.sync.dma_start(wstrip[:], wqbpT_d[:, fp * P:(fp + 1) * P]
                                  .rearrange("(a p) m -> p a m", p=P))
                pp = ps1.tile([P, 512], F32, tag="acc", name="pp")[:, :TPC]
                for k in range(QK):
                    nc.tensor.matmul(pp[:], wstrip[:, k, :], qlatT[:, k, :],
                                     start=(k == 0), stop=(k == QK - 1))
                nc.vector.tensor_copy(out=qpeT[:, fp, :], in_=pp[:])
            for tt in range(TPC // P):
                c2 = sb.tile([P, DR], F32, tag="p1c2")
                s2 = sb.tile([P, DR], F32, tag="p1s2")
                nc.sync.dma_start(c2[:], ctab2_loc[tt * P:(tt + 1) * P, :])
                nc.sync.dma_start(s2[:], stab2_loc[tt * P:(tt + 1) * P, :])
                qpe_tok = p1res.tile([P, P], F32R, tag="qpetok", bufs=2)
                for fp in range(PFC):
                    transpose_to(psT1, qpe_tok[:], qpeT[:, fp, tt * P:(tt + 1) * P], P)
                    rope_tok(qpe_tok[:], c2[:], s2[:], P, "qb")
                    tb = psT1.tile([P, P], F32R, tag="tr")
                    nc.tensor.transpose(tb[:], qpe_tok[:], ident[:])
                    tb_bf = sb.tile([P, P], BF16, tag="tbbf")
                    nc.vector.tensor_copy(out=tb_bf[:], in_=tb[:])
                    b0 = fp * DST + DN
                    b1 = fp * DST + DQK + DN
                    nc.sync.dma_start(a2aq_in[b0:b0 + DR, tt * P:(tt + 1) * P], tb_bf[:DR, :])
                    nc.sync.dma_start(a2aq_in[b1:b1 + DR, tt * P:(tt + 1) * P], tb_bf[DR:, :])

        ccq = nc.gpsimd.collective_compute(
            "AllToAll", ALU.bypass, replica_groups=rg,
            ins=[a2aq_in[:].opt()], outs=[a2aq_out[:].opt()])
        add_dep_helper(ccq.ins, cc1k.ins, sync=True, reason="serialize collectives")

        # ================= Phase 2 + 3 =================
        with tc.tile_pool(name="wop", bufs=1) as wop:
            wo_sb = wop.tile([P, HK, HID], BF16)
            for k in range(HK):
                hsrc = (2 * k) if k < 8 else (2 * (k - 8) + 1)
                nc.sync.dma_start(wo_sb[:, k, :], woT_d[hsrc * DV:(hsrc + 1) * DV, :])

            with tc.tile_pool(name="attn", bufs=1) as at, \
                 tc.tile_pool(name="ps2", bufs=4, space="PSUM") as ps2, \
                 tc.tile_pool(name="psD", bufs=1, space="PSUM") as psD, \
                 tc.tile_pool(name="psB", bufs=1, space="PSUM") as psB, \
                 tc.tile_pool(name="psT2", bufs=2, space="PSUM") as psT2:
                kpeT = at.tile([DR, T], BF16)
                for b in range(NC):
                    nc.sync.dma_start(kpeT[:, b * TPC:(b + 1) * TPC],
                                      ag1k_out[b * AGK + KVLR:b * AGK + AGK, :])
                qnT = at.tile([P, HPC, T], BF16)
                qpT = at.tile([DR, HPC, T], BF16)
                for b in range(NC):
                    for hh in range(HPC):
                        base = b * DST + hh * DQK
                        nc.sync.dma_start(qnT[:, hh, b * TPC:(b + 1) * TPC],
                                          a2aq_out[base:base + DN, :])
                        nc.sync.dma_start(qpT[:, hh, b * TPC:(b + 1) * TPC],
                                          a2aq_out[base + DN:base + DQK, :])
                wkvn_sb = at.tile([P, KK, HPC * DN], BF16)
                wkvv_sb = at.tile([P, KK, HPC * DV], BF16)
                nc.sync.dma_start(wkvn_sb[:], wkv_nope_d[:].rearrange("(a p) m -> p a m", p=P))
                nc.sync.dma_start(wkvv_sb[:], wkv_v_d[:].rearrange("(a p) m -> p a m", p=P))

                knT = at.tile([P, HPC, T], BF16)
                vtok = at.tile([P, T // P, HPC, DV], F32R)
                for bp in range(NC // 2):
                    cs = slice(bp * 2 * TPC, (bp + 1) * 2 * TPC)
                    ckv_sb = at.tile([P, KK, 2 * TPC], BF16, tag="ckv_b", bufs=1)
                    for w in range(2):
                        b = 2 * bp + w
                        nc.sync.dma_start(ckv_sb[:, :, w * TPC:(w + 1) * TPC],
                                          ag1k_out[b * AGK:b * AGK + KVLR, :]
                                          .rearrange("(a p) m -> p a m", p=P))
                    for hh in range(HPC):
                        pkn = ps2.tile([P, 2 * TPC], F32, tag="acc")
                        for k in range(KK):
                            nc.tensor.matmul(pkn[:], wkvn_sb[:, k, hh * DN:(hh + 1) * DN],
                                             ckv_sb[:, k, :], start=(k == 0), stop=(k == KK - 1))
                        nc.vector.tensor_copy(out=knT[:, hh, cs], in_=pkn[:])
                        pv = ps2.tile([P, 2 * TPC], F32, tag="acc")
                        for k in range(KK):
                            nc.tensor.matmul(pv[:], wkvv_sb[:, k, hh * DV:(hh + 1) * DV],
                                             ckv_sb[:, k, :], start=(k == 0), stop=(k == KK - 1))
                        vT_sb = at.tile([P, 2 * TPC], F32R, tag="vT_sb", bufs=2)
                        nc.vector.tensor_copy(out=vT_sb[:], in_=pv[:])
                        for tq in range(2 * TPC // P):
                            transpose_to(psT2, vtok[:, bp * 4 + tq, hh, :],
                                         vT_sb[:, tq * P:(tq + 1) * P], P)

                NQ = 512
                cc_att = []
                for hh in range(HPC):
                    a2a_dst = a2aA_in if hh == 0 else a2aB_in
                    for j in range(T // NQ):
                        qs = slice(j * NQ, (j + 1) * NQ)
                        u_ps = ps2.tile([P, NQ], F32, tag="acc")
                        den_ps = psD.tile([1, NQ], F32, tag="den")
                        n_kt = 4 * (j + 1)
                        for i in range(n_kt):
                            ks = slice(i * P, (i + 1) * P)
                            s_ps = ps2.tile([P, NQ], F32, tag="acc")
                            nc.tensor.matmul(s_ps[:], knT[:, hh, ks], qnT[:, hh, qs],
                                             start=True, stop=False)
                            nc.tensor.matmul(s_ps[:], kpeT[:, ks], qpT[:, hh, qs],
                                             start=False, stop=True)
                            e_sb = at.tile([P, NQ], F32R, tag="e_sb", bufs=3)
                            nc.scalar.activation(e_sb[:], s_ps[:], AF.Exp)
                            if i >= 4 * j:
                                nc.vector.tensor_tensor(e_sb[:], e_sb[:],
                                                        mask_sb[:, i - 4 * j, :], ALU.mult)
                            nc.tensor.matmul(den_ps[:], ones_col[:], e_sb[:],
                                             start=(i == 0), stop=(i == n_kt - 1),
                                             skip_group_check=True)
                            nc.tensor.matmul(u_ps[:], vtok[:, i, hh, :], e_sb[:],
                                             start=(i == 0), stop=(i == n_kt - 1),
                                             skip_group_check=True)
                        rden = at.tile([1, NQ], F32, tag="rden", bufs=2)
                        nc.vector.reciprocal(rden[:], den_ps[:])
                        bc_ps = psB.tile([P, NQ], F32, tag="bc")
                        nc.tensor.matmul(bc_ps[:], ones_row[:], rden[:], start=True, stop=True)
                        bc_sb = at.tile([P, NQ], F32, tag="bc_sb", bufs=2)
                        nc.vector.tensor_copy(out=bc_sb[:], in_=bc_ps[:])
                        a_sb = at.tile([P, NQ], BF16, tag="a_sb", bufs=2)
                        nc.vector.tensor_tensor(a_sb[:], u_ps[:], bc_sb[:], ALU.mult)
                        for w in range(2):
                            tw = 2 * j + w
                            nc.sync.dma_start(
                                a2a_dst[tw * DV:(tw + 1) * DV, :],
                                a_sb[:, w * TPC:(w + 1) * TPC])
                    cc = nc.gpsimd.collective_compute(
                        "AllToAll", ALU.bypass, replica_groups=rg,
                        ins=[(a2aA_in if hh == 0 else a2aB_in)[:].opt()],
                        outs=[(a2aA_out if hh == 0 else a2aB_out)[:].opt()])
                    add_dep_helper(cc.ins, (ccq if hh == 0 else cc_att[0]).ins,
                                   sync=True, reason="serialize collectives")
                    cc_att.append(cc)

            # ---- Phase 3: o_proj + residual + ln_post, chunked AG3 ----
            cc_ag3 = []
            with tc.tile_pool(name="p3", bufs=1) as p3, \
                 tc.tile_pool(name="ps3", bufs=4, space="PSUM") as ps3, \
                 tc.tile_pool(name="psT3", bufs=2, space="PSUM") as psT3:
                for tt in range(TPC // P):
                    slabA = p3.tile([P, 8, P], BF16, tag="slabA", bufs=2)
                    nc.sync.dma_start(slabA[:], a2aA_out[:, tt * P:(tt + 1) * P]
                                      .rearrange("(a p) m -> p a m", p=P))
                    slabB = p3.tile([P, 8, P], BF16, tag="slabB", bufs=2)
                    nc.sync.dma_start(slabB[:], a2aB_out[:, tt * P:(tt + 1) * P]
                                      .rearrange("(a p) m -> p a m", p=P))
                    h_sb = p3.tile([P, HID], F32, tag="h_in3", bufs=2)
                    nc.sync.dma_start(h_sb[:], h_rows[tt * P:(tt + 1) * P, :])
                    r2sb = p3.tile([P, HID], F32, tag="r2sb", bufs=2)
                    for hc in range(4):
                        po = ps3.tile([P, 512], F32, tag="acc")
                        for k in range(HK):
                            slab = slabA if k < 8 else slabB
                            nc.tensor.matmul(po[:], slab[:, k % 8, :],
                                             wo_sb[:, k, hc * 512:(hc + 1) * 512],
                                             start=(k == 0), stop=(k == HK - 1))
                        nc.vector.tensor_tensor(r2sb[:, hc * 512:(hc + 1) * 512], po[:],
                                                h_sb[:, hc * 512:(hc + 1) * 512], ALU.add)
                    nc.sync.dma_start(r2out[tt * P:(tt + 1) * P, :], r2sb[:])
                    ss = sb.tile([P, 1], F32, tag="p3ss")
                    nc.scalar.activation(sqscr[:], r2sb[:], AF.Square, accum_out=ss[:])
                    sinv = rms_scale_from(ss[:], HID, "p3")
                    h2n = p3.tile([P, HID], F32R, tag="h2n", bufs=2)
                    nc.vector.tensor_scalar_mul(h2n[:], r2sb[:], sinv[:])
                    for k in range(HK):
                        tp_ = psT3.tile([P, P], F32R, tag="tr")
                        nc.tensor.transpose(tp_[:], h2n[:, k * P:(k + 1) * P], ident[:])
                        tbf = sb.tile([P, P], BF16, tag="h2bf")
                        nc.vector.tensor_copy(out=tbf[:], in_=tp_[:])
                        nc.sync.dma_start(ag3_in_t[tt][k * P:(k + 1) * P, :], tbf[:])
                    cc = nc.gpsimd.collective_compute(
                        "AllGather", ALU.bypass, replica_groups=rg,
                        ins=[ag3_in_t[tt][:].opt()], outs=[ag3_out_t[tt][:].opt()])
                    add_dep_helper(cc.ins, (cc_att[1] if tt == 0 else cc_ag3[0]).ins,
                                   sync=True, reason="serialize collectives")
                    cc_ag3.append(cc)

        # ================= Phase 4: single-pass bf16 MLP =================
        with tc.tile_pool(name="p4", bufs=1) as p4, \
             tc.tile_pool(name="p4w", bufs=2) as p4w, \
             tc.tile_pool(name="ps4", bufs=8, space="PSUM") as ps4:
            h2T = p4.tile([P, HK, T], BF16)
            for tt in range(2):
                for b in range(NC):
                    nc.sync.dma_start(
                        h2T[:, :, b * TPC + tt * P:b * TPC + (tt + 1) * P],
                        ag3_out_t[tt][b * HID:(b + 1) * HID, :]
                        .rearrange("(a p) m -> p a m", p=P))
            wd_sb = p4.tile([P, HK, NI, P], BF16)
            for hm in range(HK):
                nc.sync.dma_start(wd_sb[:, hm, :, :], wdT_d[:, hm * P:(hm + 1) * P]
                                  .rearrange("(a p) m -> p a m", p=P))
            mT = p4.tile([P, NI, T], BF16)
            for m in range(NI):
                wg_strip = p4w.tile([P, HK, P], BF16, tag="wg")
                nc.sync.dma_start(wg_strip[:], wgT_d[:, m * P:(m + 1) * P]
                                  .rearrange("(a p) m -> p a m", p=P))
                wu_strip = p4w.tile([P, HK, P], BF16, tag="wu")
                nc.sync.dma_start(wu_strip[:], wuT_d[:, m * P:(m + 1) * P]
                                  .rearrange("(a p) m -> p a m", p=P))
                pg = [ps4.tile([P, 512], F32, tag="acc", name=f"pg{_}") for _ in range(4)]
                pu = [ps4.tile([P, 512], F32, tag="acc", name=f"pu{_}") for _ in range(4)]
                for k in range(HK):
                    for ts in range(4):
                        nc.tensor.matmul(pg[ts][:], wg_strip[:, k, :],
                                         h2T[:, k, ts * 512:(ts + 1) * 512],
                                         start=(k == 0), stop=(k == HK - 1),
                                         skip_group_check=True)
                    for ts in range(4):
                        nc.tensor.matmul(pu[ts][:], wu_strip[:, k, :],
                                         h2T[:, k, ts * 512:(ts + 1) * 512],
                                         start=(k == 0), stop=(k == HK - 1),
                                         skip_group_check=True)
                for ts in range(4):
                    sg = p4.tile([P, 512], F32, tag="sg", bufs=2)
                    nc.scalar.activation(sg[:], pg[ts][:], AF.Silu)
                    nc.vector.tensor_tensor(mT[:, m, ts * 512:(ts + 1) * 512],
                                            sg[:], pu[ts][:], ALU.mult)
            cc_prev = cc_ag3[1]
            for c in range(4):
                for hm in range(HK):
                    pd = ps4.tile([P, 512], F32, tag="acc", name="pd")
                    for m in range(NI):
                        nc.tensor.matmul(pd[:], wd_sb[:, hm, m, :],
                                         mT[:, m, c * 512:(c + 1) * 512],
                                         start=(m == 0), stop=(m == NI - 1),
                                         skip_group_check=True)
                    dsb = p4.tile([P, 512], BF16, tag="dsb", bufs=2)
                    nc.vector.tensor_copy(out=dsb[:], in_=pd[:])
                    nc.sync.dma_start(rs_in_c[c][hm * P:(hm + 1) * P, :], dsb[:])
                cc4 = nc.gpsimd.collective_compute(
                    "ReduceScatter", ALU.add, replica_groups=rg,
                    ins=[rs_in_c[c][:].opt()], outs=[rs_out_c[c][:].opt()])
                add_dep_helper(cc4.ins, cc_prev.ins, sync=True, reason="serialize collectives")
                cc_prev = cc4
                nc.sync.dma_start(mlp_part[:, c * 512:(c + 1) * 512], rs_out_c[c][:])

    nc.finalize()
    return nc


# ===================== host side =====================
import numpy as np
import ml_dtypes

BF = ml_dtypes.bfloat16

_nc_cache = None


def _get_nc():
    global _nc_cache
    if _nc_cache is None:
        _nc_cache = build()
    return _nc_cache


def prep_shards(inputs):
    f = lambda x: np.ascontiguousarray(np.asarray(x, np.float32))
    bf = lambda x: np.ascontiguousarray(np.asarray(x, np.float32).astype(BF))
    hs = f(inputs["hidden_states"])
    cos = f(inputs["cos"])[:, 0, :]
    sin = f(inputs["sin"])[:, 0, :]
    ctab = cos[:, :DR // 2]                     # (T, 32)
    stab = sin[:, :DR // 2]
    ctab2 = np.concatenate([ctab, ctab], 1)     # (T, 64) for 2-head q rope
    stab2 = np.concatenate([stab, stab], 1)
    ln_in = f(inputs["ln_input"])
    wqaT = bf((inputs["w_q_a"] * ln_in[None, :]).T)
    wkvaT = bf((inputs["w_kv_a"] * ln_in[None, :]).T)
    wqbT = f((inputs["w_q_b"] * f(inputs["ln_q_a"])[None, :]).T) * np.float32(DQK ** -0.5)
    wkvbT = f((inputs["w_kv_b"] * f(inputs["ln_kv_a"])[None, :]).T)
    woT = bf(inputs["w_o"].T)
    ln_post = f(inputs["ln_post"])
    wgT = f((inputs["w_gate"] * ln_post[None, :]).T)
    wuT = f((inputs["w_up"] * ln_post[None, :]).T)
    wdT = f(inputs["w_down"].T)

    # causal diagonal masks: variant m valid iff 128*m + kt_local <= qt_local
    masks = np.zeros((4, P, 512), np.float32)
    kt = np.arange(P)[:, None]
    qt = np.arange(512)[None, :]
    for m in range(4):
        masks[m] = (P * m + kt <= qt).astype(np.float32)
    masks = masks.reshape(4 * P, 512)

    wqb3 = wqbT.reshape(QLR, H, DQK)
    wqbnT = bf(wqb3[:, :, :DN].reshape(QLR, H * DN))
    wqbpT = bf(wqb3[:, :, DN:].reshape(QLR, H * DR))
    wkv3 = wkvbT.reshape(KVLR, H, DN + DV)
    shards = []
    for i in range(NC):
        hsl = slice(i * HPC, (i + 1) * HPC)
        wkv_nope = bf(wkv3[:, hsl, :DN].reshape(KVLR, HPC * DN))
        wkv_v = bf(wkv3[:, hsl, DN:].reshape(KVLR, HPC * DV))
        wg_s = np.zeros((HID, IPAD), np.float32)
        wg_s[:, :IPC] = wgT[:, i * IPC:(i + 1) * IPC]
        wu_s = np.zeros((HID, IPAD), np.float32)
        wu_s[:, :IPC] = wuT[:, i * IPC:(i + 1) * IPC]
        wd_s = np.zeros((IPAD, HID), np.float32)
        wd_s[:IPC, :] = wdT[i * IPC:(i + 1) * IPC, :]
        tsl = slice(i * TPC, (i + 1) * TPC)
        shards.append({
            "h_rows": np.ascontiguousarray(hs[tsl]),
            "ctab_loc": np.ascontiguousarray(ctab[tsl]),
            "stab_loc": np.ascontiguousarray(stab[tsl]),
            "ctab2_loc": np.ascontiguousarray(ctab2[tsl]),
            "stab2_loc": np.ascontiguousarray(stab2[tsl]),
            "wqaT": wqaT, "wkvaT": wkvaT,
            "wqbnT": wqbnT, "wqbpT": wqbpT,
            "wkv_nope": wkv_nope, "wkv_v": wkv_v,
            "woT": woT, "wgT": bf(wg_s), "wuT": bf(wu_s), "wdT": bf(wd_s),
            "masks": masks,
        })
    return shards


def kernel(**inputs):
    from concourse.bass_utils import run_bass_kernel_spmd
    nc = _get_nc()
    shards = prep_shards(inputs)
    res = run_bass_kernel_spmd(nc, shards, core_ids=list(range(NC)))
    return assemble(res.results)


def assemble(results):
    r2 = np.concatenate([results[i]["r2out"] for i in range(NC)], axis=0)      # (T, HID)
    mlpT = np.concatenate([results[i]["mlp_part"] for i in range(NC)], axis=0).astype(np.float32)  # (HID, T)
    return r2 + mlpT.T
